# revision 10
# baseline (speedup 1.0000x reference)
"""Trainium2 Bass kernel for BiDAF-style bidirectional attention.

Reference computation (per batch element n; M=1 folded away):
    s[i,j]  = h[i].w_h + u[j].w_u + (h[i]*u[j]).w_hu + b      [JX, JQ]
    a_u     = softmax_j(s);     u_a[i] = sum_j a_u[i,j] u[j]   (c2q)
    a_h     = softmax_i(max_j s);  h_a = sum_i a_h[i] h[i]     (q2c)
    out     = concat(h, u_a, h*u_a, h*h_a)                     [JX, 4D]

Sharding: data-parallel over batch N=8, one NeuronCore per batch element.
alpha_b drops out entirely (both softmaxes are shift-invariant).

Key algebra: w_h folds into the score weights: with
    uw'[j,d] = u[j,d]*w_hu[d] + w_h[d],
    sT[j,i]  = sum_d uw'[j,d] h[i,d]  =  (h.w_h)[i] + ((h*w_hu).u)[i,j],
so ET = exp(sT + uwu[j]) = exp(s - b) exactly: no separate h.w_h pass.
m_exp[i] = max_j ET (exact q2c logits), rowsum_j ET = c2q denominator.

HW model this schedule is built around (measured):
  - each DMA instruction costs ~600 ns of SEQUENCER issue time (fixed) and
    drains on ONE hardware queue at ~26 GB/s (2-4KB descriptors);
  - three engines can issue DMAs: sync + scalar (HWDGE) and gpsimd (SWDGE);
  - per-engine instruction streams execute IN ORDER at runtime, so every
    stream is emitted in expected-readiness order;
  - PE clock ramps with continuous execution (~3 us to full speed) -> small
    f32r warmup burst first; f32r matmuls with N>=512 run 1 cycle/row;
    PE transposes with the f32r identity as moving operand: 1.5 cycles/row.
DMA plan: h split into 16 half-tile (128KB) loads spread over the three
issue engines so ~10 queues drain in parallel; u split x2; alpha_w loaded
once ([1,1536], one descriptor) and broadcast across partitions ON-CHIP via
three K=1 PE matmuls (a broadcast DMA would be 128 descriptors on one
queue, ~25 us).  Outputs: stg (cols 1-2) split x2 per tile on sync, o4
(col 3) on scalar, h passthrough (col 0) on gpsimd gated behind block-0's
exp so it fills the post-load DMA window instead of racing the h loads.

dtype scheme: tensors feeding f32r matmuls are TYPED f32r at their producer
(DMA loads via source bitcast; DVE/ACT writers emit f32r) satisfying the
walrus FP32r-rounding verifier with no conversion copies; f32 consumers
read the same bits via .bitcast(f32).
"""

import numpy as np

N_B, M_B, JX, JQ, D = 8, 1, 1024, 128, 512
P = 128
NT = JX // P   # 8 i-tiles
KC = D // P    # 4 d-chunks
IB = 512       # i-block width for score matmuls
NB = JX // IB  # 2 blocks
TPB = NT // NB  # tiles per block
HP = P // 2    # h DMA piece height (half tile)

_CACHE = {}


def _build_program():
    from contextlib import ExitStack

    import concourse.bass as bass
    import concourse.tile as tile
    from concourse import bacc, mybir
    from concourse.masks import make_identity
    from concourse.tile_rust import add_dep_helper

    f32 = mybir.dt.float32
    f32r = mybir.dt.float32r
    EXP = mybir.ActivationFunctionType.Exp
    AX = mybir.AxisListType.X
    ds = bass.ds

    nc = bacc.Bacc("TRN2", target_bir_lowering=False, debug=False, num_devices=8)
    h_d = nc.dram_tensor("h", [JX, D], f32, kind="ExternalInput").ap()
    u_d = nc.dram_tensor("u", [JQ, D], f32, kind="ExternalInput").ap()
    aw_d = nc.dram_tensor("alpha_w", [3 * D], f32, kind="ExternalInput").ap()
    out_d = nc.dram_tensor("out", [JX, 4 * D], f32, kind="ExternalOutput").ap()

    with tile.TileContext(nc) as tc, ExitStack() as ctx:
        consts = ctx.enter_context(tc.tile_pool(name="consts", bufs=1))
        stage = ctx.enter_context(tc.tile_pool(name="stage", bufs=6))
        # PSUM banks: tp=2, s0=2, ua=2, acc=1 (warmup/et_b0/zqp/bc), hap=1
        ps = ctx.enter_context(tc.tile_pool(name="ps", bufs=2, space="PSUM"))

        # ---- input DMAs: h as 16 half-tile pieces across the three issue
        # engines (each engine's pieces in tile order); aw + u on scalar.
        h_all = consts.tile([P, NT * D], f32r)   # tile t: h[t*128+p, d]
        u_sb = consts.tile([JQ, D], f32r)
        aw_sb = consts.tile([1, 3 * D], f32r)

        def h_piece(eng, t, half):
            r0 = t * P + half * HP
            eng.dma_start(
                h_all[ds(half * HP, HP), ds(t * D, D)],
                h_d[ds(r0, HP), :].bitcast(f32r),
            )

        for t in (0,):                      # scalar: tile 0 first
            h_piece(nc.scalar, t, 0), h_piece(nc.scalar, t, 1)
        nc.scalar.dma_start(aw_sb[:], aw_d.rearrange("(o d) -> o d", o=1).bitcast(f32r))
        for half in (0, 1):                 # u split x2 on scalar
            nc.scalar.dma_start(
                u_sb[ds(half * HP, HP), :], u_d[ds(half * HP, HP), :].bitcast(f32r)
            )
        for t in (1, 2, 3, 4, 5):           # sync: tiles 1-5
            h_piece(nc.sync, t, 0), h_piece(nc.sync, t, 1)
        for t in (6, 7):                    # gpsimd: tiles 6-7
            h_piece(nc.gpsimd, t, 0), h_piece(nc.gpsimd, t, 1)

        h_f = h_all[:].bitcast(f32)
        u_f = u_sb[:].bitcast(f32)

        # ---- PE warmup: f32r matmuls depending only on DVE memset+copy,
        # emitted early so the clock-gate ramp starts while h DMAs stream in.
        warm_f = consts.tile([P, D], f32)
        nc.vector.memset(warm_f[:], 0.25)
        warm = consts.tile([P, D], f32r)
        nc.vector.tensor_copy(warm[:], warm_f[:])
        wp = ps.tile([P, D], f32, tag="acc", bufs=1)
        for w in range(8):
            nc.tensor.matmul(
                wp[:], warm[:, ds(0, P)], warm[:], start=True, stop=True,
            )

        # ---- constants.  ones_row's scalar.copy is the first ACT op: it
        # triggers the 1.3us ACT table load early, under the h loads.
        ones_row_f = consts.tile([1, P], f32)
        nc.vector.memset(ones_row_f[:], 1.0)
        ones_row = consts.tile([1, P], f32r)
        nc.scalar.copy(ones_row[:], ones_row_f[:])
        ones_col = consts.tile([P, 1], f32)
        nc.vector.memset(ones_col[:], 1.0)
        ident_f = consts.tile([P, P], f32)
        make_identity(nc, ident_f[:])
        ident = consts.tile([P, P], f32r)
        nc.vector.tensor_copy(ident[:], ident_f[:])

        # ---- alpha_w broadcast on-chip: wcast[p, c*D+d] = aw[c*D+d] via
        # K=1 matmuls (ones_row x aw chunk) into s0-tag PSUM; the DVE uw'
        # chain reads the PSUM tiles directly (no eviction).
        w_ps = []
        for c in range(3):
            wt = ps.tile([P, D], f32, tag="s0")
            nc.tensor.matmul(
                wt[:], ones_row[:], aw_sb[:, ds(c * D, D)], start=True, stop=True
            )
            w_ps.append(wt)
        wh_p, wu_p, whu_p = w_ps

        # uw[j,d] = u[j,d]*w_hu[d] + w_h[d];  uwu[j] = sum_d u[j,d]*w_u[d]
        uw = consts.tile([JQ, D], f32r)
        uw0 = consts.tile([JQ, D], f32)
        nc.vector.tensor_mul(uw0[:], u_f, whu_p[:])
        nc.vector.tensor_add(uw[:], uw0[:], wh_p[:])
        uwtmp = consts.tile([JQ, D], f32)
        uwu = consts.tile([JQ, 1], f32)
        nc.vector.scalar_tensor_tensor(
            uwtmp[:], u_f, 1.0, wu_p[:],
            op0=mybir.AluOpType.mult, op1=mybir.AluOpType.mult, accum_out=uwu[:],
        )

        # ---- hT transposes (4 per tile into tp PSUM, batched evictions
        # split Scalar/DVE) interleaved with uwT; emission follows expected
        # h-piece arrival order.
        hT_all = consts.tile([P, KC * JX], f32r)  # chunk k: hT[k*128+p, i]
        hT3 = hT_all[:].rearrange("p (k x) -> p k x", k=KC)

        def transpose_tile(t):
            pt = ps.tile([P, KC * P], f32r, tag="tp")
            for k in range(KC):
                nc.tensor.transpose(
                    pt[:, ds(k * P, P)], h_all[:, ds(t * D + k * P, P)], ident[:]
                )
            ev = nc.scalar.copy if t % 2 == 0 else nc.vector.tensor_copy
            ev(hT3[:, :, ds(t * P, P)], pt[:].rearrange("p (k x) -> p k x", k=KC))

        uwT = consts.tile([P, KC * JQ], f32r)

        transpose_tile(0)
        transpose_tile(1)
        ptw = ps.tile([P, KC * P], f32r, tag="tp")
        for k in range(KC):
            nc.tensor.transpose(ptw[:, ds(k * P, P)], uw[:, ds(k * P, P)], ident[:])
        nc.scalar.copy(uwT[:], ptw[:])
        transpose_tile(2)
        transpose_tile(3)

        # ---- block-0 scores + exp ----
        ET = consts.tile([JQ, JX], f32r)          # exp(sT + uwu[j]) = exp(s - b)
        m_exp = consts.tile([P, NT], f32r)        # per i-tile: max_j ET
        z_rec = consts.tile([P, NT], f32)         # per i-tile: 1/sum_j ET

        def scores_block(b):
            sp = ps.tile([JQ, IB], f32, tag="s0")
            for k in range(KC):
                nc.tensor.matmul(
                    sp[:], uwT[:, ds(k * JQ, JQ)], hT_all[:, ds(k * JX + b * IB, IB)],
                    start=(k == 0), stop=(k == KC - 1),
                )
            # ET = exp(sT + uwu[j]); uwu is the per-partition (j) ACT bias
            return nc.scalar.activation(
                ET[:, ds(b * IB, IB)], sp[:], EXP, bias=uwu[:]
            )

        def retrans_reduce(b, tag):
            et = ps.tile([P, TPB * P], f32r, tag=tag, bufs=1 if tag == "acc" else 2)
            for q in range(TPB):
                t = b * TPB + q
                nc.tensor.transpose(et[:, ds(q * P, P)], ET[:, ds(t * P, P)], ident[:])
            et3 = et[:].rearrange("p (q x) -> p q x", q=TPB)
            nc.vector.reduce_max(m_exp[:, ds(b * TPB, TPB)], et3, axis=AX)
            zsum = stage.tile([P, TPB], f32, tag="zs")
            nc.vector.reduce_sum(zsum[:], et3, axis=AX)
            nc.vector.reciprocal(z_rec[:, ds(b * TPB, TPB)], zsum[:])

        exp_b0 = scores_block(0)
        # h passthrough (col 0): gpsimd queues, gated behind block-0's exp
        # so the 2MB lands in the mid-kernel DMA window, not under h loads.
        for t in range(NT):
            ho = nc.gpsimd.dma_start(
                out_d[ds(t * P, P), ds(0, D)].bitcast(f32r), h_all[:, ds(t * D, D)]
            )
            add_dep_helper(ho.ins, exp_b0.ins, sync=True,
                           reason="delay h passthrough into DMA lull")

        transpose_tile(4)
        transpose_tile(5)
        retrans_reduce(0, "acc")
        transpose_tile(6)
        transpose_tile(7)

        # ---- c2q for block-0 tiles: u_a matmul, normalize, write cols 1-2
        # (split x2 on sync queues)
        def c2q_tile(t):
            up = ps.tile([P, D], f32, tag="ua")
            nc.tensor.matmul(
                up[:], ET[:, ds(t * P, P)], u_sb[:], start=True, stop=True
            )
            stg = stage.tile([P, 2 * D], f32, tag="stg")
            nc.scalar.mul(stg[:, ds(0, D)], up[:], z_rec[:, ds(t, 1)])
            nc.vector.scalar_tensor_tensor(
                stg[:, ds(D, D)], up[:], z_rec[:, ds(t, 1)], h_f[:, ds(t * D, D)],
                op0=mybir.AluOpType.mult, op1=mybir.AluOpType.mult,
            )
            for half in (0, 1):
                nc.sync.dma_start(
                    out_d[ds(t * P + half * HP, HP), ds(D, 2 * D)],
                    stg[ds(half * HP, HP), :],
                )

        for t in range(TPB):
            c2q_tile(t)

        exp_b1 = scores_block(1)
        retrans_reduce(1, "tp")

        # ---- q2c: hap += m_exp[i] h[i] (contiguous K=1 group), total,
        # broadcast back; bc lands before the block-1 c2q tiles stream out.
        hap = ps.tile([1, D], f32, tag="hap", bufs=1)
        for t in range(NT):
            nc.tensor.matmul(
                hap[:], m_exp[:, ds(t, 1)], h_all[:, ds(t * D, D)],
                start=(t == 0), stop=(t == NT - 1),
                skip_group_check=True,
            )
        mrow = consts.tile([P, 1], f32)
        nc.vector.reduce_sum(mrow[:], m_exp[:].bitcast(f32), axis=AX)
        zqp = ps.tile([1, 1], f32, tag="acc", bufs=1)
        nc.tensor.matmul(zqp[:], mrow[:], ones_col[:], start=True, stop=True)
        rzq = consts.tile([1, 1], f32)
        nc.vector.reciprocal(rzq[:], zqp[:])
        ha_sum = consts.tile([1, D], f32)
        nc.vector.tensor_copy(ha_sum[:], hap[:])
        ha_row = consts.tile([1, D], f32r)
        nc.scalar.mul(ha_row[:], ha_sum[:], rzq[:])
        bc = ps.tile([P, D], f32, tag="acc", bufs=1)
        nc.tensor.matmul(bc[:], ones_row[:], ha_row[:], start=True, stop=True)
        bc_sb = consts.tile([P, D], f32)
        nc.scalar.copy(bc_sb[:], bc[:])

        # ---- block-1 c2q + o4 (col 3) for all tiles: muls split DVE/GpSimd,
        # writes on scalar queues.
        def o4_tile(t):
            o4 = stage.tile([P, D], f32, tag="o4")
            mul = nc.vector.tensor_mul if t % 2 == 0 else nc.gpsimd.tensor_mul
            mul(o4[:], h_f[:, ds(t * D, D)], bc_sb[:])
            nc.scalar.dma_start(out_d[ds(t * P, P), ds(3 * D, D)], o4[:])

        for q in range(TPB):
            c2q_tile(TPB + q)
            o4_tile(2 * q)
            o4_tile(2 * q + 1)

    nc.compile()
    return nc


def _get_nc():
    if "nc" not in _CACHE:
        _CACHE["nc"] = _build_program()
    return _CACHE["nc"]


def _ensure_axon_hooks_stub():
    # concourse imports antenv.axon_hooks when tracing is requested via env;
    # provide a no-op stub if the image lacks it so runs degrade gracefully.
    import sys
    import types

    try:
        import antenv.axon_hooks  # noqa: F401
    except ImportError:
        mod = types.ModuleType("antenv.axon_hooks")
        _hook = [None]
        mod.set_axon_ntff_profile_hook = lambda hook: _hook.__setitem__(0, hook)
        mod.get_axon_ntff_profile_hook = lambda: _hook[0]
        sys.modules["antenv.axon_hooks"] = mod


def kernel(h, u, alpha_w, alpha_b=None, **_unused):
    _ensure_axon_hooks_stub()
    from concourse.bass_utils import run_bass_kernel_spmd

    h = np.ascontiguousarray(np.asarray(h, dtype=np.float32)).reshape(N_B, JX, D)
    u = np.ascontiguousarray(np.asarray(u, dtype=np.float32)).reshape(N_B, JQ, D)
    alpha_w = np.ascontiguousarray(np.asarray(alpha_w, dtype=np.float32)).reshape(3 * D)

    nc = _get_nc()
    in_maps = [
        {"h": h[n], "u": u[n], "alpha_w": alpha_w} for n in range(N_B)
    ]
    res = run_bass_kernel_spmd(nc, in_maps, core_ids=list(range(N_B)))
    out = np.stack([res.results[n]["out"] for n in range(N_B)], axis=0)
    return out.reshape(N_B, M_B, JX, 4 * D)


# revision 11
# speedup vs baseline: 1.0233x; 1.0233x over previous
"""Trainium2 Bass kernel for BiDAF-style bidirectional attention.

Reference computation (per batch element n; M=1 folded away):
    s[i,j]  = h[i].w_h + u[j].w_u + (h[i]*u[j]).w_hu + b      [JX, JQ]
    a_u     = softmax_j(s);     u_a[i] = sum_j a_u[i,j] u[j]   (c2q)
    a_h     = softmax_i(max_j s);  h_a = sum_i a_h[i] h[i]     (q2c)
    out     = concat(h, u_a, h*u_a, h*h_a)                     [JX, 4D]

Sharding: data-parallel over batch N=8, one NeuronCore per batch element.
alpha_b drops out entirely (both softmaxes are shift-invariant).

Key algebra: with uw'[j,d] = u[j,d]*w_hu[d] + w_h[d],
    sT[j,i] = sum_d uw'[j,d] h[i,d] = (h.w_h)[i] + ((h*w_hu).u)[i,j],
so ET = exp(sT + uwu[j]) = exp(s - b) exactly; m_exp[i] = max_j ET gives
exact q2c logits and rowsum_j ET the c2q denominators.

HW model this schedule is built around (measured on-trace):
  - each DMA instruction costs ~600 ns of SEQUENCER issue time (fixed) and
    drains on ONE hardware queue at ~26 GB/s; ~16 queues total; issuing
    with >16 DMAs in flight credit-stalls the issuing engine;
  - three engines issue DMAs: sync + scalar (HWDGE), gpsimd (SWDGE);
  - per-engine instruction streams execute IN ORDER at runtime -> emit
    every stream in expected-readiness order;
  - HBM cap ~358 GB/s; total I/O 10.65 MB -> ~30 us saturated floor.
Schedule: u split x4 (sync 2 + scalar 2, first) -> uw' chain early; alpha_w
loaded once ([1,1536]) and partition-broadcast ON-CHIP via K=1 matmuls
(whu first - it gates uw'); h split into half-tile 128KB pieces spread
sync/scalar/gpsimd; scores in 4 blocks of 256 columns so the first c2q
writes start ~2 us earlier; h passthrough per tile on gpsimd as loads
retire; q2c chain right after the last reduce; tiles 0-3 write cols 1-2
(split x2) + col 3 separately, tiles 4-7 (ready after bc anyway) write
cols 1-3 as ONE fused 6KB-row DMA per half tile.  All output writes issue
on sync; scalar keeps only its 5 input issues + ACT work.

dtype scheme: tensors feeding f32r matmuls are TYPED f32r at the producer
(DMA loads via source bitcast; DVE/ACT writers emit f32r) so the walrus
FP32r-rounding verifier passes with no conversion copies; f32 consumers
read the same bits via .bitcast(f32).
"""

import numpy as np

N_B, M_B, JX, JQ, D = 8, 1, 1024, 128, 512
P = 128
NT = JX // P   # 8 i-tiles
KC = D // P    # 4 d-chunks
IB = 256       # i-block width for score matmuls
NB = JX // IB  # 4 blocks
TPB = NT // NB  # 2 tiles per block
HP = P // 2    # DMA piece height (half tile)

_CACHE = {}


def _build_program():
    from contextlib import ExitStack

    import concourse.bass as bass
    import concourse.tile as tile
    from concourse import bacc, mybir
    from concourse.masks import make_identity

    f32 = mybir.dt.float32
    f32r = mybir.dt.float32r
    EXP = mybir.ActivationFunctionType.Exp
    AX = mybir.AxisListType.X
    ds = bass.ds

    nc = bacc.Bacc("TRN2", target_bir_lowering=False, debug=False, num_devices=8)
    h_d = nc.dram_tensor("h", [JX, D], f32, kind="ExternalInput").ap()
    u_d = nc.dram_tensor("u", [JQ, D], f32, kind="ExternalInput").ap()
    aw_d = nc.dram_tensor("alpha_w", [3 * D], f32, kind="ExternalInput").ap()
    out_d = nc.dram_tensor("out", [JX, 4 * D], f32, kind="ExternalOutput").ap()

    with tile.TileContext(nc) as tc, ExitStack() as ctx:
        consts = ctx.enter_context(tc.tile_pool(name="consts", bufs=1))
        stage = ctx.enter_context(tc.tile_pool(name="stage", bufs=6))
        # PSUM banks: tp=2, s0=2, ua=2, acc=1 (warmup/zqp/bc), hap=1
        ps = ctx.enter_context(tc.tile_pool(name="ps", bufs=2, space="PSUM"))

        h_all = consts.tile([P, NT * D], f32r)   # tile t: h[t*128+p, d]
        u_sb = consts.tile([JQ, D], f32r)
        aw_sb = consts.tile([1, 3 * D], f32r)

        UQ = JQ // 4

        def h_piece(eng, t, half):
            r0 = t * P + half * HP
            eng.dma_start(
                h_all[ds(half * HP, HP), ds(t * D, D)],
                h_d[ds(r0, HP), :].bitcast(f32r),
            )

        # sync: u quarters 0-1, then h tiles 1-5 first halves
        for q in (0, 1):
            nc.sync.dma_start(
                u_sb[ds(q * UQ, UQ), :], u_d[ds(q * UQ, UQ), :].bitcast(f32r)
            )
        for t in (1, 2, 3, 4, 5):
            h_piece(nc.sync, t, 0)
        # scalar: u quarters 2-3, alpha_w row, h tile 0
        for q in (2, 3):
            nc.scalar.dma_start(
                u_sb[ds(q * UQ, UQ), :], u_d[ds(q * UQ, UQ), :].bitcast(f32r)
            )
        nc.scalar.dma_start(aw_sb[:], aw_d.rearrange("(o d) -> o d", o=1).bitcast(f32r))
        h_piece(nc.scalar, 0, 0)
        h_piece(nc.scalar, 0, 1)
        # gpsimd: h tiles 1-5 second halves, tiles 6-7 whole
        for t in (1, 2, 3):
            h_piece(nc.gpsimd, t, 1)
        for t in (6, 7):
            nc.gpsimd.dma_start(
                h_all[:, ds(t * D, D)], h_d[ds(t * P, P), :].bitcast(f32r)
            )
        for t in (4, 5):
            h_piece(nc.gpsimd, t, 1)

        h_f = h_all[:].bitcast(f32)
        u_f = u_sb[:].bitcast(f32)

        # ---- PE warmup: f32r matmuls dependent only on DVE memset+copy,
        # keeps PE continuously busy so the clock ramps while h streams in.
        warm_f = consts.tile([P, D], f32)
        nc.vector.memset(warm_f[:], 0.25)
        warm = consts.tile([P, D], f32r)
        nc.vector.tensor_copy(warm[:], warm_f[:])
        wp = ps.tile([P, D], f32, tag="acc", bufs=1)
        for w in range(10):
            nc.tensor.matmul(
                wp[:], warm[:, ds(0, P)], warm[:], start=True, stop=True,
            )

        # ---- constants.  ones_row's scalar.copy is the first ACT op: it
        # triggers the 1.5us ACT table load early, under the h loads.
        ones_row_f = consts.tile([1, P], f32)
        nc.vector.memset(ones_row_f[:], 1.0)
        ones_row = consts.tile([1, P], f32r)
        nc.scalar.copy(ones_row[:], ones_row_f[:])
        ones_col = consts.tile([P, 1], f32)
        nc.vector.memset(ones_col[:], 1.0)
        ident_f = consts.tile([P, P], f32)
        make_identity(nc, ident_f[:])
        ident = consts.tile([P, P], f32r)
        nc.vector.tensor_copy(ident[:], ident_f[:])

        # ---- alpha_w partition-broadcast on-chip via K=1 matmuls into
        # s0-tag PSUM; the DVE uw' chain reads PSUM directly.  whu FIRST
        # (it gates uw' mul), then wh (gates add), then wu (gates uwu).
        def wcast(c):
            wt = ps.tile([P, D], f32, tag="s0")
            nc.tensor.matmul(
                wt[:], ones_row[:], aw_sb[:, ds(c * D, D)], start=True, stop=True
            )
            return wt

        whu_p = wcast(2)
        wh_p = wcast(0)

        # uw[j,d] = u[j,d]*w_hu[d] + w_h[d];  uwu[j] = sum_d u[j,d]*w_u[d]
        uw = consts.tile([JQ, D], f32r)
        uw0 = consts.tile([JQ, D], f32)
        nc.vector.tensor_mul(uw0[:], u_f, whu_p[:])
        nc.vector.tensor_add(uw[:], uw0[:], wh_p[:])
        wu_p = wcast(1)
        uwtmp = consts.tile([JQ, D], f32)
        uwu = consts.tile([JQ, 1], f32)
        nc.vector.scalar_tensor_tensor(
            uwtmp[:], u_f, 1.0, wu_p[:],
            op0=mybir.AluOpType.mult, op1=mybir.AluOpType.mult, accum_out=uwu[:],
        )

        # ---- hT transposes (4 per tile into tp PSUM, batched evictions
        # split Scalar/DVE) + h passthrough per tile on gpsimd as the loads
        # retire.
        hT_all = consts.tile([P, KC * JX], f32r)  # chunk k: hT[k*128+p, i]
        hT3 = hT_all[:].rearrange("p (k x) -> p k x", k=KC)

        def transpose_tile(t):
            nc.gpsimd.dma_start(
                out_d[ds(t * P, P), ds(0, D)].bitcast(f32r), h_all[:, ds(t * D, D)]
            )
            pt = ps.tile([P, KC * P], f32r, tag="tp")
            for k in range(KC):
                nc.tensor.transpose(
                    pt[:, ds(k * P, P)], h_all[:, ds(t * D + k * P, P)], ident[:]
                )
            ev = nc.scalar.copy if t % 2 == 0 else nc.vector.tensor_copy
            ev(hT3[:, :, ds(t * P, P)], pt[:].rearrange("p (k x) -> p k x", k=KC))

        ET = consts.tile([JQ, JX], f32r)          # exp(sT + uwu[j]) = exp(s - b)
        m_exp = consts.tile([P, NT], f32r)        # per i-tile: max_j ET
        z_rec = consts.tile([P, NT], f32)         # per i-tile: 1/sum_j ET

        def scores_block(b):
            sp = ps.tile([JQ, IB], f32, tag="s0")
            for k in range(KC):
                nc.tensor.matmul(
                    sp[:], uwT[:, ds(k * JQ, JQ)], hT_all[:, ds(k * JX + b * IB, IB)],
                    start=(k == 0), stop=(k == KC - 1),
                )
            # ET = exp(sT + uwu[j]); uwu is the per-partition (j) ACT bias
            nc.scalar.activation(ET[:, ds(b * IB, IB)], sp[:], EXP, bias=uwu[:])

        def retrans_reduce(b):
            et = ps.tile([P, TPB * P], f32r, tag="tp")
            for q in range(TPB):
                t = b * TPB + q
                nc.tensor.transpose(et[:, ds(q * P, P)], ET[:, ds(t * P, P)], ident[:])
            et3 = et[:].rearrange("p (q x) -> p q x", q=TPB)
            nc.vector.reduce_max(m_exp[:, ds(b * TPB, TPB)], et3, axis=AX)
            zsum = stage.tile([P, TPB], f32, tag="zs")
            nc.vector.reduce_sum(zsum[:], et3, axis=AX)
            nc.vector.reciprocal(z_rec[:, ds(b * TPB, TPB)], zsum[:])

        # u_a matmul + col-1 normalize + col-2 product for tile t.
        # col2_eng: 'dve' = stt from PSUM; 'gps' = gpsimd col1*h from SBUF.
        stgs = {}

        def c2q_tile(t, col2_eng, width):
            up = ps.tile([P, D], f32, tag="ua")
            nc.tensor.matmul(
                up[:], ET[:, ds(t * P, P)], u_sb[:], start=True, stop=True
            )
            stg = stage.tile([P, width * D], f32, tag=f"st{width}")
            stgs[t] = stg
            nc.scalar.mul(stg[:, ds(0, D)], up[:], z_rec[:, ds(t, 1)])
            if col2_eng == "dve":
                nc.vector.scalar_tensor_tensor(
                    stg[:, ds(D, D)], up[:], z_rec[:, ds(t, 1)], h_f[:, ds(t * D, D)],
                    op0=mybir.AluOpType.mult, op1=mybir.AluOpType.mult,
                )
            else:
                nc.gpsimd.tensor_mul(
                    stg[:, ds(D, D)], stg[:, ds(0, D)], h_f[:, ds(t * D, D)]
                )

        def write_stg(t):                 # cols 1-2, two half-tile DMAs
            for half in (0, 1):
                nc.sync.dma_start(
                    out_d[ds(t * P + half * HP, HP), ds(D, 2 * D)],
                    stgs[t][ds(half * HP, HP), :],
                )

        def write_fused(t):               # cols 1-3, two 6KB-row DMAs
            for half in (0, 1):
                nc.sync.dma_start(
                    out_d[ds(t * P + half * HP, HP), ds(D, 3 * D)],
                    stgs[t][ds(half * HP, HP), :],
                )

        # ---- PE-stream spine, in expected-readiness order ----
        transpose_tile(0)
        transpose_tile(1)
        uwT = consts.tile([P, KC * JQ], f32r)
        ptw = ps.tile([P, KC * P], f32r, tag="tp")
        for k in range(KC):
            nc.tensor.transpose(ptw[:, ds(k * P, P)], uw[:, ds(k * P, P)], ident[:])
        nc.scalar.copy(uwT[:], ptw[:])

        scores_block(0)
        transpose_tile(2)
        retrans_reduce(0)
        transpose_tile(3)
        scores_block(1)
        transpose_tile(4)
        retrans_reduce(1)
        transpose_tile(5)
        scores_block(2)
        c2q_tile(0, "dve", 2)
        write_stg(0)
        c2q_tile(1, "dve", 2)
        write_stg(1)
        transpose_tile(6)
        retrans_reduce(2)
        transpose_tile(7)
        scores_block(3)
        c2q_tile(2, "dve", 2)
        write_stg(2)
        c2q_tile(3, "dve", 2)
        write_stg(3)
        retrans_reduce(3)

        # ---- q2c: hap += m_exp[i] h[i] (contiguous K=1 group), total,
        # broadcast back
        hap = ps.tile([1, D], f32, tag="hap", bufs=1)
        for t in range(NT):
            nc.tensor.matmul(
                hap[:], m_exp[:, ds(t, 1)], h_all[:, ds(t * D, D)],
                start=(t == 0), stop=(t == NT - 1),
                skip_group_check=True,
            )
        mrow = consts.tile([P, 1], f32)
        nc.vector.reduce_sum(mrow[:], m_exp[:].bitcast(f32), axis=AX)
        zqp = ps.tile([1, 1], f32, tag="acc", bufs=1)
        nc.tensor.matmul(zqp[:], mrow[:], ones_col[:], start=True, stop=True)
        rzq = consts.tile([1, 1], f32)
        nc.vector.reciprocal(rzq[:], zqp[:])
        ha_sum = consts.tile([1, D], f32)
        nc.vector.tensor_copy(ha_sum[:], hap[:])
        ha_row = consts.tile([1, D], f32r)
        nc.scalar.mul(ha_row[:], ha_sum[:], rzq[:])
        bc = ps.tile([P, D], f32, tag="acc", bufs=1)
        nc.tensor.matmul(bc[:], ones_row[:], ha_row[:], start=True, stop=True)
        bc_sb = consts.tile([P, D], f32)
        nc.scalar.copy(bc_sb[:], bc[:])

        # ---- o4 for tiles 0-3 (separate col-3 writes, muls DVE/GpSimd),
        # fused col1-3 for tiles 4-7 (col 3 written into the staging tile).
        def o4_tile(t):
            o4 = stage.tile([P, D], f32, tag="o4")
            mul = nc.vector.tensor_mul if t % 2 == 0 else nc.gpsimd.tensor_mul
            mul(o4[:], h_f[:, ds(t * D, D)], bc_sb[:])
            nc.sync.dma_start(out_d[ds(t * P, P), ds(3 * D, D)], o4[:])

        for q in range(TPB * 2):
            t = 4 + q
            c2q_tile(t, "gps", 3)
            mul = nc.vector.tensor_mul if t % 2 == 0 else nc.gpsimd.tensor_mul
            mul(stgs[t][:, ds(2 * D, D)], h_f[:, ds(t * D, D)], bc_sb[:])
            write_fused(t)
            o4_tile(q)

    nc.compile()
    return nc


def _get_nc():
    if "nc" not in _CACHE:
        _CACHE["nc"] = _build_program()
    return _CACHE["nc"]


def _ensure_axon_hooks_stub():
    # concourse imports antenv.axon_hooks when tracing is requested via env;
    # provide a no-op stub if the image lacks it so runs degrade gracefully.
    import sys
    import types

    try:
        import antenv.axon_hooks  # noqa: F401
    except ImportError:
        mod = types.ModuleType("antenv.axon_hooks")
        _hook = [None]
        mod.set_axon_ntff_profile_hook = lambda hook: _hook.__setitem__(0, hook)
        mod.get_axon_ntff_profile_hook = lambda: _hook[0]
        sys.modules["antenv.axon_hooks"] = mod


def kernel(h, u, alpha_w, alpha_b=None, **_unused):
    _ensure_axon_hooks_stub()
    from concourse.bass_utils import run_bass_kernel_spmd

    h = np.ascontiguousarray(np.asarray(h, dtype=np.float32)).reshape(N_B, JX, D)
    u = np.ascontiguousarray(np.asarray(u, dtype=np.float32)).reshape(N_B, JQ, D)
    alpha_w = np.ascontiguousarray(np.asarray(alpha_w, dtype=np.float32)).reshape(3 * D)

    nc = _get_nc()
    in_maps = [
        {"h": h[n], "u": u[n], "alpha_w": alpha_w} for n in range(N_B)
    ]
    res = run_bass_kernel_spmd(nc, in_maps, core_ids=list(range(N_B)))
    out = np.stack([res.results[n]["out"] for n in range(N_B)], axis=0)
    return out.reshape(N_B, M_B, JX, 4 * D)


# revision 12
# speedup vs baseline: 1.1202x; 1.0947x over previous
"""Trainium2 Bass kernel for BiDAF-style bidirectional attention.

Reference computation (per batch element n; M=1 folded away):
    s[i,j]  = h[i].w_h + u[j].w_u + (h[i]*u[j]).w_hu + b      [JX, JQ]
    a_u     = softmax_j(s);     u_a[i] = sum_j a_u[i,j] u[j]   (c2q)
    a_h     = softmax_i(max_j s);  h_a = sum_i a_h[i] h[i]     (q2c)
    out     = concat(h, u_a, h*u_a, h*h_a)                     [JX, 4D]

Sharding: data-parallel over batch N=8, one NeuronCore per batch element.
alpha_b drops out entirely (both softmaxes are shift-invariant).

Key algebra: with uw'[j,d] = u[j,d]*w_hu[d] + w_h[d],
    sT[j,i] = sum_d uw'[j,d] h[i,d] = (h.w_h)[i] + ((h*w_hu).u)[i,j],
so ET = exp(sT + uwu[j]) = exp(s - b) exactly; m_exp[i] = max_j ET gives
exact q2c logits and rowsum_j ET the c2q denominators.

HW model this schedule is built around (measured on-trace):
  - each DMA instruction costs ~600 ns of SEQUENCER issue time (fixed) and
    drains on ONE hardware queue at ~26 GB/s; ~16 queues total; issuing
    with >16 DMAs in flight credit-stalls the issuing engine;
  - three engines issue DMAs: sync + scalar (HWDGE), gpsimd (SWDGE);
  - per-engine instruction streams execute IN ORDER at runtime -> emit
    every stream in expected-readiness order;
  - HBM cap ~358 GB/s; total I/O 10.65 MB -> ~30 us saturated floor.
Schedule: u split x4 (sync 2 + scalar 2, first) -> uw' chain early; alpha_w
loaded once ([1,1536]) and partition-broadcast ON-CHIP via K=1 matmuls
(whu first - it gates uw'); h split into half-tile 128KB pieces spread
sync/scalar/gpsimd; scores in 4 blocks of 256 columns so the first c2q
writes start ~2 us earlier; h passthrough per tile on gpsimd as loads
retire; q2c chain right after the last reduce; tiles 0-3 write cols 1-2
(split x2) + col 3 separately, tiles 4-7 (ready after bc anyway) write
cols 1-3 as ONE fused 6KB-row DMA per half tile.  All output writes issue
on sync; scalar keeps only its 5 input issues + ACT work.

dtype scheme: tensors feeding f32r matmuls are TYPED f32r at the producer
(DMA loads via source bitcast; DVE/ACT writers emit f32r) so the walrus
FP32r-rounding verifier passes with no conversion copies; f32 consumers
read the same bits via .bitcast(f32).
"""

import numpy as np

N_B, M_B, JX, JQ, D = 8, 1, 1024, 128, 512
P = 128
NT = JX // P   # 8 i-tiles
KC = D // P    # 4 d-chunks
IB = 256       # i-block width for score matmuls
NB = JX // IB  # 4 blocks
TPB = NT // NB  # 2 tiles per block
HP = P // 2    # DMA piece height (half tile)

_CACHE = {}


def _build_program():
    from contextlib import ExitStack

    import concourse.bass as bass
    import concourse.tile as tile
    from concourse import bacc, mybir
    from concourse.masks import make_identity

    f32 = mybir.dt.float32
    f32r = mybir.dt.float32r
    EXP = mybir.ActivationFunctionType.Exp
    AX = mybir.AxisListType.X
    ds = bass.ds

    nc = bacc.Bacc("TRN2", target_bir_lowering=False, debug=False, num_devices=8)
    h_d = nc.dram_tensor("h", [JX, D], f32, kind="ExternalInput").ap()
    u_d = nc.dram_tensor("u", [JQ, D], f32, kind="ExternalInput").ap()
    aw_d = nc.dram_tensor("alpha_w", [3 * D], f32, kind="ExternalInput").ap()
    out_d = nc.dram_tensor("out", [JX, 4 * D], f32, kind="ExternalOutput").ap()

    with tile.TileContext(nc) as tc, ExitStack() as ctx:
        consts = ctx.enter_context(tc.tile_pool(name="consts", bufs=1))
        stage = ctx.enter_context(tc.tile_pool(name="stage", bufs=6))
        # PSUM banks: tp=2, s0=2, ua=2, acc=1 (warmup/zqp/bc), hap=1
        ps = ctx.enter_context(tc.tile_pool(name="ps", bufs=2, space="PSUM"))

        h_all = consts.tile([P, NT * D], f32r)   # tile t: h[t*128+p, d]
        u_sb = consts.tile([JQ, D], f32r)
        aw_sb = consts.tile([1, 3 * D], f32r)

        UQ = JQ // 4

        def h_piece(eng, t, part, npieces):
            rows = P // npieces
            r0 = t * P + part * rows
            eng.dma_start(
                h_all[ds(part * rows, rows), ds(t * D, D)],
                h_d[ds(r0, rows), :].bitcast(f32r),
            )

        # The earliest tiles gate the whole score pipeline: tiles 0-1 are
        # quartered (64KB pieces, ~2.5us queue drain each) split sync/gpsimd,
        # tiles 2-7 are halved (sync gets half a, gpsimd half b).
        for t in (0, 1):                    # sync: quarters 0-1
            h_piece(nc.sync, t, 0, 4), h_piece(nc.sync, t, 1, 4)
        for t in (2, 3, 4, 5, 6, 7):
            h_piece(nc.sync, t, 0, 2)
        for q in range(4):                  # scalar: u quarters, then aw
            nc.scalar.dma_start(
                u_sb[ds(q * UQ, UQ), :], u_d[ds(q * UQ, UQ), :].bitcast(f32r)
            )
        nc.scalar.dma_start(aw_sb[:], aw_d.rearrange("(o d) -> o d", o=1).bitcast(f32r))
        for t in (0, 1):                    # gpsimd: quarters 2-3
            h_piece(nc.gpsimd, t, 2, 4), h_piece(nc.gpsimd, t, 3, 4)
        for t in (2, 3, 4, 5, 6, 7):
            h_piece(nc.gpsimd, t, 1, 2)

        h_f = h_all[:].bitcast(f32)
        u_f = u_sb[:].bitcast(f32)

        # ---- PE warmup: f32r matmuls dependent only on DVE memset+copy,
        # keeps PE continuously busy so the clock ramps while h streams in.
        warm_f = consts.tile([P, D], f32)
        nc.vector.memset(warm_f[:], 0.25)
        warm = consts.tile([P, D], f32r)
        nc.vector.tensor_copy(warm[:], warm_f[:])
        wp = ps.tile([P, D], f32, tag="acc", bufs=1)
        for w in range(10):
            nc.tensor.matmul(
                wp[:], warm[:, ds(0, P)], warm[:], start=True, stop=True,
            )

        # ---- constants.  ones_row's scalar.copy is the first ACT op: it
        # triggers the 1.5us ACT table load early, under the h loads.
        ones_row_f = consts.tile([1, P], f32)
        nc.vector.memset(ones_row_f[:], 1.0)
        ones_row = consts.tile([1, P], f32r)
        nc.scalar.copy(ones_row[:], ones_row_f[:])
        ones_col = consts.tile([P, 1], f32)
        nc.vector.memset(ones_col[:], 1.0)
        ident_f = consts.tile([P, P], f32)
        make_identity(nc, ident_f[:])
        ident = consts.tile([P, P], f32r)
        nc.vector.tensor_copy(ident[:], ident_f[:])

        # ---- alpha_w partition-broadcast on-chip via K=1 matmuls into
        # s0-tag PSUM; the DVE uw' chain reads PSUM directly.  whu FIRST
        # (it gates uw' mul), then wh (gates add), then wu (gates uwu).
        def wcast(c):
            wt = ps.tile([P, D], f32, tag="s0")
            nc.tensor.matmul(
                wt[:], ones_row[:], aw_sb[:, ds(c * D, D)], start=True, stop=True
            )
            return wt

        whu_p = wcast(2)
        wh_p = wcast(0)

        # uw[j,d] = u[j,d]*w_hu[d] + w_h[d];  uwu[j] = sum_d u[j,d]*w_u[d]
        uw = consts.tile([JQ, D], f32r)
        uw0 = consts.tile([JQ, D], f32)
        nc.vector.tensor_mul(uw0[:], u_f, whu_p[:])
        nc.vector.tensor_add(uw[:], uw0[:], wh_p[:])
        wu_p = wcast(1)
        uwtmp = consts.tile([JQ, D], f32)
        uwu = consts.tile([JQ, 1], f32)
        nc.vector.scalar_tensor_tensor(
            uwtmp[:], u_f, 1.0, wu_p[:],
            op0=mybir.AluOpType.mult, op1=mybir.AluOpType.mult, accum_out=uwu[:],
        )

        # ---- hT transposes (4 per tile into tp PSUM, batched evictions
        # split Scalar/DVE) + h passthrough per tile on gpsimd as the loads
        # retire.
        hT_all = consts.tile([P, KC * JX], f32r)  # chunk k: hT[k*128+p, i]
        hT3 = hT_all[:].rearrange("p (k x) -> p k x", k=KC)

        def transpose_tile(t):
            nc.gpsimd.dma_start(
                out_d[ds(t * P, P), ds(0, D)].bitcast(f32r), h_all[:, ds(t * D, D)]
            )
            pt = ps.tile([P, KC * P], f32r, tag="tp")
            for k in range(KC):
                nc.tensor.transpose(
                    pt[:, ds(k * P, P)], h_all[:, ds(t * D + k * P, P)], ident[:]
                )
            ev = nc.scalar.copy if t % 2 == 0 else nc.vector.tensor_copy
            ev(hT3[:, :, ds(t * P, P)], pt[:].rearrange("p (k x) -> p k x", k=KC))

        ET = consts.tile([JQ, JX], f32r)          # exp(sT + uwu[j]) = exp(s - b)
        m_exp = consts.tile([P, NT], f32r)        # per i-tile: max_j ET
        z_rec = consts.tile([P, NT], f32)         # per i-tile: 1/sum_j ET

        def scores_block(b):
            sp = ps.tile([JQ, IB], f32, tag="s0")
            for k in range(KC):
                nc.tensor.matmul(
                    sp[:], uwT[:, ds(k * JQ, JQ)], hT_all[:, ds(k * JX + b * IB, IB)],
                    start=(k == 0), stop=(k == KC - 1),
                )
            # ET = exp(sT + uwu[j]); uwu is the per-partition (j) ACT bias
            nc.scalar.activation(ET[:, ds(b * IB, IB)], sp[:], EXP, bias=uwu[:])

        def retrans_reduce(b):
            et = ps.tile([P, TPB * P], f32r, tag="tp")
            for q in range(TPB):
                t = b * TPB + q
                nc.tensor.transpose(et[:, ds(q * P, P)], ET[:, ds(t * P, P)], ident[:])
            et3 = et[:].rearrange("p (q x) -> p q x", q=TPB)
            nc.vector.reduce_max(m_exp[:, ds(b * TPB, TPB)], et3, axis=AX)
            zsum = stage.tile([P, TPB], f32, tag="zs")
            nc.vector.reduce_sum(zsum[:], et3, axis=AX)
            nc.vector.reciprocal(z_rec[:, ds(b * TPB, TPB)], zsum[:])

        # u_a matmul + col-1 normalize + col-2 product for tile t.
        # col2_eng: 'dve' = stt from PSUM; 'gps' = gpsimd col1*h from SBUF.
        stgs = {}

        def c2q_tile(t, col2_eng, width):
            up = ps.tile([P, D], f32, tag="ua")
            nc.tensor.matmul(
                up[:], ET[:, ds(t * P, P)], u_sb[:], start=True, stop=True
            )
            stg = stage.tile([P, width * D], f32, tag=f"st{width}")
            stgs[t] = stg
            nc.scalar.mul(stg[:, ds(0, D)], up[:], z_rec[:, ds(t, 1)])
            if col2_eng == "dve":
                nc.vector.scalar_tensor_tensor(
                    stg[:, ds(D, D)], up[:], z_rec[:, ds(t, 1)], h_f[:, ds(t * D, D)],
                    op0=mybir.AluOpType.mult, op1=mybir.AluOpType.mult,
                )
            else:
                nc.gpsimd.tensor_mul(
                    stg[:, ds(D, D)], stg[:, ds(0, D)], h_f[:, ds(t * D, D)]
                )

        def write_stg(t):                 # cols 1-2, two half-tile DMAs
            for half in (0, 1):
                nc.sync.dma_start(
                    out_d[ds(t * P + half * HP, HP), ds(D, 2 * D)],
                    stgs[t][ds(half * HP, HP), :],
                )

        def write_fused(t):               # cols 1-3, two 6KB-row DMAs
            for half in (0, 1):
                nc.sync.dma_start(
                    out_d[ds(t * P + half * HP, HP), ds(D, 3 * D)],
                    stgs[t][ds(half * HP, HP), :],
                )

        # ---- PE-stream spine, in expected-readiness order ----
        transpose_tile(0)
        transpose_tile(1)
        uwT = consts.tile([P, KC * JQ], f32r)
        ptw = ps.tile([P, KC * P], f32r, tag="tp")
        for k in range(KC):
            nc.tensor.transpose(ptw[:, ds(k * P, P)], uw[:, ds(k * P, P)], ident[:])
        nc.scalar.copy(uwT[:], ptw[:])

        scores_block(0)
        transpose_tile(2)
        retrans_reduce(0)
        transpose_tile(3)
        scores_block(1)
        transpose_tile(4)
        retrans_reduce(1)
        transpose_tile(5)
        scores_block(2)
        c2q_tile(0, "dve", 2)
        write_stg(0)
        c2q_tile(1, "dve", 2)
        write_stg(1)
        transpose_tile(6)
        retrans_reduce(2)
        transpose_tile(7)
        scores_block(3)
        c2q_tile(2, "dve", 2)
        write_stg(2)
        c2q_tile(3, "dve", 2)
        write_stg(3)
        retrans_reduce(3)

        # ---- q2c: hap += m_exp[i] h[i] (contiguous K=1 group), total,
        # broadcast back
        hap = ps.tile([1, D], f32, tag="hap", bufs=1)
        for t in range(NT):
            nc.tensor.matmul(
                hap[:], m_exp[:, ds(t, 1)], h_all[:, ds(t * D, D)],
                start=(t == 0), stop=(t == NT - 1),
                skip_group_check=True,
            )
        mrow = consts.tile([P, 1], f32)
        nc.vector.reduce_sum(mrow[:], m_exp[:].bitcast(f32), axis=AX)
        zqp = ps.tile([1, 1], f32, tag="acc", bufs=1)
        nc.tensor.matmul(zqp[:], mrow[:], ones_col[:], start=True, stop=True)
        rzq = consts.tile([1, 1], f32)
        nc.vector.reciprocal(rzq[:], zqp[:])
        ha_sum = consts.tile([1, D], f32)
        nc.vector.tensor_copy(ha_sum[:], hap[:])
        ha_row = consts.tile([1, D], f32r)
        nc.scalar.mul(ha_row[:], ha_sum[:], rzq[:])
        bc = ps.tile([P, D], f32, tag="acc", bufs=1)
        nc.tensor.matmul(bc[:], ones_row[:], ha_row[:], start=True, stop=True)
        bc_sb = consts.tile([P, D], f32)
        nc.scalar.copy(bc_sb[:], bc[:])

        # ---- o4 for tiles 0-3 (separate col-3 writes, muls DVE/GpSimd),
        # fused col1-3 for tiles 4-7 (col 3 written into the staging tile).
        def o4_tile(t):
            o4 = stage.tile([P, D], f32, tag="o4")
            mul = nc.vector.tensor_mul if t % 2 == 0 else nc.gpsimd.tensor_mul
            mul(o4[:], h_f[:, ds(t * D, D)], bc_sb[:])
            nc.sync.dma_start(out_d[ds(t * P, P), ds(3 * D, D)], o4[:])

        for q in range(TPB * 2):
            t = 4 + q
            c2q_tile(t, "gps", 3)
            mul = nc.vector.tensor_mul if t % 2 == 0 else nc.gpsimd.tensor_mul
            mul(stgs[t][:, ds(2 * D, D)], h_f[:, ds(t * D, D)], bc_sb[:])
            write_fused(t)
            o4_tile(q)

    nc.compile()
    return nc


def _get_nc():
    if "nc" not in _CACHE:
        _CACHE["nc"] = _build_program()
    return _CACHE["nc"]


def _ensure_axon_hooks_stub():
    # concourse imports antenv.axon_hooks when tracing is requested via env;
    # provide a no-op stub if the image lacks it so runs degrade gracefully.
    import sys
    import types

    try:
        import antenv.axon_hooks  # noqa: F401
    except ImportError:
        mod = types.ModuleType("antenv.axon_hooks")
        _hook = [None]
        mod.set_axon_ntff_profile_hook = lambda hook: _hook.__setitem__(0, hook)
        mod.get_axon_ntff_profile_hook = lambda: _hook[0]
        sys.modules["antenv.axon_hooks"] = mod


def kernel(h, u, alpha_w, alpha_b=None, **_unused):
    _ensure_axon_hooks_stub()
    from concourse.bass_utils import run_bass_kernel_spmd

    h = np.ascontiguousarray(np.asarray(h, dtype=np.float32)).reshape(N_B, JX, D)
    u = np.ascontiguousarray(np.asarray(u, dtype=np.float32)).reshape(N_B, JQ, D)
    alpha_w = np.ascontiguousarray(np.asarray(alpha_w, dtype=np.float32)).reshape(3 * D)

    nc = _get_nc()
    in_maps = [
        {"h": h[n], "u": u[n], "alpha_w": alpha_w} for n in range(N_B)
    ]
    res = run_bass_kernel_spmd(nc, in_maps, core_ids=list(range(N_B)))
    out = np.stack([res.results[n]["out"] for n in range(N_B)], axis=0)
    return out.reshape(N_B, M_B, JX, 4 * D)


# revision 13
# speedup vs baseline: 1.2454x; 1.1117x over previous
"""Trainium2 Bass kernel for BiDAF-style bidirectional attention.

Reference computation (per batch element n; M=1 folded away):
    s[i,j]  = h[i].w_h + u[j].w_u + (h[i]*u[j]).w_hu + b      [JX, JQ]
    a_u     = softmax_j(s);     u_a[i] = sum_j a_u[i,j] u[j]   (c2q)
    a_h     = softmax_i(max_j s);  h_a = sum_i a_h[i] h[i]     (q2c)
    out     = concat(h, u_a, h*u_a, h*h_a)                     [JX, 4D]

Sharding: data-parallel over batch N=8, one NeuronCore per batch element.
alpha_b drops out entirely (both softmaxes are shift-invariant).

Key algebra vs the straightforward mapping:
  - w_h folds into the score weights: with uw'[j,d] = u[j,d]*w_hu[d]+w_h[d],
    sT[j,i] = sum_d uw'[j,d] h[i,d] = (h.w_h)[i] + ((h*w_hu).u)[i,j], so
    ET = exp(sT + uwu[j]) = exp(s - b) exactly and the whole h.w_h pass
    (PSUM row accumulators + evictions) disappears.
  - alpha_w is loaded once as a [1,1536] row (one descriptor) and partition-
    broadcast ON-CHIP via three K=1 matmuls into PSUM which the DVE reads
    directly.  (A [P,3D] broadcast DMA is 128 x 6KB descriptors on one
    queue: ~20+ us, and it sat on the critical path to the scores.)
  - f32r-at-source dtype scheme: tensors feeding f32r matmuls are TYPED
    f32r at their producer (DMA loads via source bitcast, DVE/ACT writers
    emit f32r) so the walrus FP32r verifier passes with no conversion
    copies; f32 consumers read the same bits via .bitcast(f32).  This
    deletes ~5 us of ScalarE h_r/u_r/ident copies the old version needed.
  - all PE transposes use the f32r identity as the moving operand
    (1.5 cycles/row instead of 2.0 for f32).

DMA structure (kept deliberately coarse: each DMA instruction costs ~0.6us
sequencer issue + ~0.9us completion semaphore, and all queues share one
~300GB/s bus, so few big transfers beat many small ones):
  - h: 8 whole-tile loads + u + aw on sync;
  - h passthrough (col 0) per tile on gpsimd, tiles >= 4 gated on block-0's
    exp so they land in the mid-kernel DMA window;
  - stg (cols 1-2, one [P,2D] 4KB-row DMA per tile) + o4 (col 3) on sync.
Per-core dataflow: PE warmup opens the clock gate under the h loads; hT via
32 PE transposes (evictions split Scalar/DVE); scores per 512-block (4
K=128 matmuls, ScalarE Exp evict with bias=uwu); ET re-transposed for DVE
max/sum reduces; q2c chain right after block-1 reduce so o4 = h*h_a writes
spread out instead of cramming at the end.
"""

import numpy as np

N_B, M_B, JX, JQ, D = 8, 1, 1024, 128, 512
P = 128
NT = JX // P   # 8 i-tiles
KC = D // P    # 4 d-chunks
IB = 512       # i-block width for score matmuls
NB = JX // IB  # 2 blocks
TPB = NT // NB  # tiles per block

_CACHE = {}


def _build_program():
    from contextlib import ExitStack

    import concourse.bass as bass
    import concourse.tile as tile
    from concourse import bacc, mybir
    from concourse.masks import make_identity
    from concourse.tile_rust import add_dep_helper

    f32 = mybir.dt.float32
    f32r = mybir.dt.float32r
    EXP = mybir.ActivationFunctionType.Exp
    AX = mybir.AxisListType.X
    ds = bass.ds

    nc = bacc.Bacc("TRN2", target_bir_lowering=False, debug=False, num_devices=8)
    h_d = nc.dram_tensor("h", [JX, D], f32, kind="ExternalInput").ap()
    u_d = nc.dram_tensor("u", [JQ, D], f32, kind="ExternalInput").ap()
    aw_d = nc.dram_tensor("alpha_w", [3 * D], f32, kind="ExternalInput").ap()
    out_d = nc.dram_tensor("out", [JX, 4 * D], f32, kind="ExternalOutput").ap()

    with tile.TileContext(nc) as tc, ExitStack() as ctx:
        consts = ctx.enter_context(tc.tile_pool(name="consts", bufs=1))
        stage = ctx.enter_context(tc.tile_pool(name="stage", bufs=6))
        # PSUM budget (8 banks): tp=2, s0=2, ua=2, acc=1, hap=1
        ps = ctx.enter_context(tc.tile_pool(name="ps", bufs=2, space="PSUM"))

        # ---- PE warmup: f32r matmuls depending only on DVE ops, emitted
        # first so the HAM clock-gate opens (1.2 -> 2.4 GHz) while the h
        # DMAs stream in.
        warm_f = consts.tile([P, D], f32)
        nc.vector.memset(warm_f[:], 0.25)
        warm = consts.tile([P, D], f32r)
        nc.vector.tensor_copy(warm[:], warm_f[:])
        wp = ps.tile([P, D], f32, tag="acc", bufs=1)
        for w in range(16):
            nc.tensor.matmul(
                wp[:], warm[:, ds(0, P)], warm[:], start=True, stop=True,
            )

        # ---- constants / prep ----
        ident_f = consts.tile([P, P], f32)
        make_identity(nc, ident_f[:])
        ident = consts.tile([P, P], f32r)
        nc.vector.tensor_copy(ident[:], ident_f[:])
        ones_row_f = consts.tile([1, P], f32)
        nc.vector.memset(ones_row_f[:], 1.0)
        ones_row = consts.tile([1, P], f32r)
        nc.scalar.copy(ones_row[:], ones_row_f[:])
        ones_col = consts.tile([P, 1], f32)
        nc.vector.memset(ones_col[:], 1.0)

        u_sb = consts.tile([JQ, D], f32r)
        nc.sync.dma_start(u_sb[:], u_d[:].bitcast(f32r))
        u_f = u_sb[:].bitcast(f32)
        aw_sb = consts.tile([1, 3 * D], f32r)
        nc.sync.dma_start(aw_sb[:], aw_d.rearrange("(o d) -> o d", o=1).bitcast(f32r))

        # alpha_w partition-broadcast on-chip: K=1 matmuls into s0 PSUM,
        # read directly by the DVE.  whu first (it gates the uw' multiply).
        def wcast(c):
            wt = ps.tile([P, D], f32, tag="s0")
            nc.tensor.matmul(
                wt[:], ones_row[:], aw_sb[:, ds(c * D, D)], start=True, stop=True
            )
            return wt

        whu_p = wcast(2)
        wh_p = wcast(0)

        # uw[j,d] = u[j,d]*w_hu[d] + w_h[d];  uwu[j] = sum_d u[j,d]*w_u[d]
        uw = consts.tile([JQ, D], f32r)
        uw0 = consts.tile([JQ, D], f32)
        nc.vector.tensor_mul(uw0[:], u_f, whu_p[:])
        nc.vector.tensor_add(uw[:], uw0[:], wh_p[:])
        wu_p = wcast(1)
        uwtmp = consts.tile([JQ, D], f32)
        uwu = consts.tile([JQ, 1], f32)
        nc.vector.scalar_tensor_tensor(
            uwtmp[:], u_f, 1.0, wu_p[:],
            op0=mybir.AluOpType.mult, op1=mybir.AluOpType.mult, accum_out=uwu[:],
        )

        # uwT[d_chunk][j]: 4 transposes into one PSUM bank, one batched evict
        uwT = consts.tile([P, KC * JQ], f32r)
        pt = ps.tile([P, KC * P], f32r, tag="tp")
        for k in range(KC):
            nc.tensor.transpose(pt[:, ds(k * P, P)], uw[:, ds(k * P, P)], ident[:])
        nc.scalar.copy(uwT[:], pt[:])

        # ---- load h; passthrough out1; build hT ----
        h_all = consts.tile([P, NT * D], f32r)    # tile t: h[t*128+p, d]
        h_f = h_all[:].bitcast(f32)
        hT_all = consts.tile([P, KC * JX], f32r)  # chunk k: hT[k*128+p, i]
        hT3 = hT_all[:].rearrange("p (k x) -> p k x", k=KC)
        hout_late = []
        for t in range(NT):
            nc.sync.dma_start(
                h_all[:, ds(t * D, D)], h_d[ds(t * P, P), :].bitcast(f32r)
            )
            # out1 = h passthrough (GpSimd DMA queue; Sync stays free).  The
            # later tiles are gated on block-0's exp (below) so ~1 MB of
            # passthrough lands in the mid-kernel DMA lull instead of
            # competing with the h loads.
            ho = nc.gpsimd.dma_start(
                out_d[ds(t * P, P), ds(0, D)].bitcast(f32r), h_all[:, ds(t * D, D)]
            )
            if t >= NT // 2:
                hout_late.append(ho)

        for t in range(NT):
            pt = ps.tile([P, KC * P], f32r, tag="tp")
            for k in range(KC):
                nc.tensor.transpose(
                    pt[:, ds(k * P, P)], h_all[:, ds(t * D + k * P, P)], ident[:]
                )
            ev = nc.scalar.copy if t % 2 == 0 else nc.vector.tensor_copy
            ev(hT3[:, :, ds(t * P, P)], pt[:].rearrange("p (k x) -> p k x", k=KC))

        # ---- scores (transposed), exp, c2q, per-block q2c accumulation ----
        ET = consts.tile([JQ, JX], f32r)          # exp(sT + uwu[j]) = exp(s - b)
        m_exp = consts.tile([P, NT], f32r)        # per i-tile: max_j ET
        z_rec = consts.tile([P, NT], f32)         # per i-tile: 1/sum_j ET
        hap = ps.tile([1, D], f32, tag="hap", bufs=1)

        for b in range(NB):
            sp = ps.tile([JQ, IB], f32, tag="s0")
            for k in range(KC):
                nc.tensor.matmul(
                    sp[:], uwT[:, ds(k * JQ, JQ)], hT_all[:, ds(k * JX + b * IB, IB)],
                    start=(k == 0), stop=(k == KC - 1),
                )
            # ET = exp(sT + uwu[j]); uwu is the per-partition (j) ACT bias
            exp_inst = nc.scalar.activation(
                ET[:, ds(b * IB, IB)], sp[:], EXP, bias=uwu[:]
            )
            if b == 0:
                for ho in hout_late:
                    add_dep_helper(ho.ins, exp_inst.ins, sync=True,
                                   reason="delay h passthrough into DMA lull")

            # re-transpose ET (4 tiles into one bank); batched 3D reduces
            et = ps.tile([P, TPB * P], f32r, tag="tp")
            for q in range(TPB):
                t = b * TPB + q
                nc.tensor.transpose(
                    et[:, ds(q * P, P)], ET[:, ds(t * P, P)], ident[:]
                )
            et3 = et[:].rearrange("p (q x) -> p q x", q=TPB)
            nc.vector.reduce_max(m_exp[:, ds(b * TPB, TPB)], et3, axis=AX)
            zsum = stage.tile([P, TPB], f32, tag="zs")
            nc.vector.reduce_sum(zsum[:], et3, axis=AX)
            nc.vector.reciprocal(z_rec[:, ds(b * TPB, TPB)], zsum[:])

            # q2c accumulation for this block's tiles (single PSUM group
            # spanning both blocks; other matmuls interleave freely)
            for q in range(TPB):
                t = b * TPB + q
                nc.tensor.matmul(
                    hap[:], m_exp[:, ds(t, 1)], h_all[:, ds(t * D, D)],
                    start=(b == 0 and q == 0), stop=(b == NB - 1 and q == TPB - 1),
                    skip_group_check=True,
                )
            if b == NB - 1:
                # q2c chain emitted ahead of the last c2q loop: bc becomes
                # ready while stg work still streams, shortening the tail
                mrow = consts.tile([P, 1], f32)
                nc.vector.reduce_sum(mrow[:], m_exp[:].bitcast(f32), axis=AX)
                zqp = ps.tile([1, 1], f32, tag="acc", bufs=1)
                nc.tensor.matmul(zqp[:], mrow[:], ones_col[:], start=True, stop=True)
                rzq = consts.tile([1, 1], f32)
                nc.vector.reciprocal(rzq[:], zqp[:])
                ha_sum = consts.tile([1, D], f32)
                nc.vector.tensor_copy(ha_sum[:], hap[:])
                ha_row = consts.tile([1, D], f32r)
                nc.scalar.mul(ha_row[:], ha_sum[:], rzq[:])
                bc = ps.tile([P, D], f32, tag="acc", bufs=1)
                nc.tensor.matmul(bc[:], ones_row[:], ha_row[:], start=True, stop=True)

            for q in range(TPB):
                t = b * TPB + q
                up = ps.tile([P, D], f32, tag="ua")
                nc.tensor.matmul(
                    up[:], ET[:, ds(t * P, P)], u_sb[:], start=True, stop=True
                )
                stg = stage.tile([P, 2 * D], f32, tag="stg")
                nc.scalar.mul(stg[:, ds(0, D)], up[:], z_rec[:, ds(t, 1)])
                nc.vector.scalar_tensor_tensor(
                    stg[:, ds(D, D)], up[:], z_rec[:, ds(t, 1)], h_f[:, ds(t * D, D)],
                    op0=mybir.AluOpType.mult, op1=mybir.AluOpType.mult,
                )
                nc.sync.dma_start(out_d[ds(t * P, P), ds(D, 2 * D)], stg[:])
                if b == NB - 1:
                    # interleave o4 tiles after each stg tile
                    for tt in range(q * (NT // TPB), (q + 1) * (NT // TPB)):
                        o4 = stage.tile([P, D], f32, tag="o4")
                        nc.vector.tensor_mul(o4[:], h_f[:, ds(tt * D, D)], bc[:])
                        nc.sync.dma_start(out_d[ds(tt * P, P), ds(3 * D, D)], o4[:])

    nc.compile()
    return nc


def _get_nc():
    if "nc" not in _CACHE:
        _CACHE["nc"] = _build_program()
    return _CACHE["nc"]


def _ensure_axon_hooks_stub():
    # concourse imports antenv.axon_hooks when tracing is requested via env;
    # provide a no-op stub if the image lacks it so runs degrade gracefully.
    import sys
    import types

    try:
        import antenv.axon_hooks  # noqa: F401
    except ImportError:
        mod = types.ModuleType("antenv.axon_hooks")
        _hook = [None]
        mod.set_axon_ntff_profile_hook = lambda hook: _hook.__setitem__(0, hook)
        mod.get_axon_ntff_profile_hook = lambda: _hook[0]
        sys.modules["antenv.axon_hooks"] = mod


def kernel(h, u, alpha_w, alpha_b=None, **_unused):
    _ensure_axon_hooks_stub()
    from concourse.bass_utils import run_bass_kernel_spmd

    h = np.ascontiguousarray(np.asarray(h, dtype=np.float32)).reshape(N_B, JX, D)
    u = np.ascontiguousarray(np.asarray(u, dtype=np.float32)).reshape(N_B, JQ, D)
    alpha_w = np.ascontiguousarray(np.asarray(alpha_w, dtype=np.float32)).reshape(3 * D)

    nc = _get_nc()
    in_maps = [
        {"h": h[n], "u": u[n], "alpha_w": alpha_w} for n in range(N_B)
    ]
    res = run_bass_kernel_spmd(nc, in_maps, core_ids=list(range(N_B)))
    out = np.stack([res.results[n]["out"] for n in range(N_B)], axis=0)
    return out.reshape(N_B, M_B, JX, 4 * D)
